# revision 24
# baseline (speedup 1.0000x reference)
"""Trainium2 Bass kernel for nn_DepthNet (multi-view depth regression).

Strategy
--------
The reference computes, per depth plane d, a homography warp of two source
views onto the reference view, a 3-view variance volume over 40 channels,
a 1x1 channel regression to a cost volume [D,H,W], then softmax over D and
depth/uncertainty regression.

Host analysis (float64) verifies that for the given projection matrices the
warp is a pure x-shift per depth plane (py == y, px == x + s_d).  Under that
(host-checked) structure each plane's warp is two shifted-window reads of the
source image, so the whole cost volume is built from dense strided reads --
no gather hardware is needed.

Math: with d1 = ref - warp1, d2 = ref - warp2 (per channel),
    9 * var_c = d1^2 + d2^2 + (d2-d1)^2 = 2*(d1 - d2/2)^2 + 1.5*d2^2
    cost = sum_c w_c var_c
Channels are pre-scaled by sqrt(2*|w_c|/9) and permuted so positive-w
channels are contiguous; the weighted channel sum becomes two plain
reductions (positive block minus negative block).

Sharding: each of the 8 cores builds 4 depth planes of the cost volume at
full [128 h-partitions] width, an AllToAll exchanges (d-shard -> h-shard),
and each core finishes softmax/depth/variance for its 16 rows.

If the structure checks fail the kernel falls back to a numpy replication of
the reference (correct, host-side).
"""

import os
import sys

import numpy as np

for _p in ("/opt/trn_rl_repo",):
    if _p not in sys.path:
        sys.path.insert(0, _p)

import concourse.bass as bass
import concourse.mybir as mybir
import concourse.tile as tile
from concourse import bacc
from concourse.bass_utils import run_bass_kernel_spmd

N_CORES = 8
LAMB = 1.5
LAST_RESULT = None  # BassKernelResults of the most recent device run
_LAST_IN_MAPS = None  # per-core input maps of the most recent device run

# build-time tunables
IMG_DT = mybir.dt.bfloat16  # dtype of the (scaled) images in SBUF
WORK_DT = mybir.dt.float32  # dtype of the per-plane difference fields
NQ = 4  # w-splits per plane (window free size = (W/NQ)*C)


# --------------------------------------------------------------------------
# host-side structure analysis
# --------------------------------------------------------------------------

def _analyze_shifts(proj_matrices, depth_values, H, W):
    """Return per-view (k[D] int, frac[D] float) if the warp is a pure
    x-shift per depth plane (within 1e-4), else None."""
    P = np.asarray(proj_matrices, np.float64)[0]  # [V,4,4]
    D = depth_values.shape[1]
    try:
        refinv = np.linalg.inv(P[0])
    except np.linalg.LinAlgError:
        return None
    dep = np.asarray(depth_values, np.float64)[0]  # [D,H,W]
    ys, xs = np.meshgrid(
        np.arange(H, dtype=np.float64), np.arange(W, dtype=np.float64),
        indexing="ij")
    out = []
    for v in range(1, P.shape[0]):
        pr = P[v] @ refinv
        rot, tr = pr[:3, :3], pr[:3, 3]
        X = rot[0, 0] * xs + rot[0, 1] * ys + rot[0, 2]
        Y = rot[1, 0] * xs + rot[1, 1] * ys + rot[1, 2]
        Z = rot[2, 0] * xs + rot[2, 1] * ys + rot[2, 2]
        den = Z[None] * dep + tr[2]
        if np.min(np.abs(den)) < 1e-6:
            return None
        px = (X[None] * dep + tr[0]) / den
        py = (Y[None] * dep + tr[1]) / den
        if np.max(np.abs(py - ys[None])) > 1e-4:
            return None
        sx = px - xs[None]  # [D,H,W]
        s_d = np.median(sx.reshape(D, -1), axis=1)
        if np.max(np.abs(sx - s_d[:, None, None])) > 1e-4:
            return None
        k = np.floor(s_d).astype(np.int64)
        out.append((k, (s_d - k).astype(np.float64)))
    return out


# --------------------------------------------------------------------------
# numpy fallback (reference replication)
# --------------------------------------------------------------------------

def _bilinear_np(img, x, y):
    c, Hs, Ws = img.shape
    x0 = np.floor(x)
    y0 = np.floor(y)
    x0i = x0.astype(np.int32)
    y0i = y0.astype(np.int32)
    wx = x - x0
    wy = y - y0
    imgf = img.reshape(c, Hs * Ws)

    def gather(xi, yi, w):
        valid = (xi >= 0) & (xi < Ws) & (yi >= 0) & (yi < Hs)
        lin = np.clip(yi, 0, Hs - 1) * Ws + np.clip(xi, 0, Ws - 1)
        v = imgf[:, lin.reshape(-1)].reshape((c,) + xi.shape)
        return v * (w * valid.astype(w.dtype))[None]

    return (gather(x0i, y0i, (1 - wx) * (1 - wy))
            + gather(x0i + 1, y0i, wx * (1 - wy))
            + gather(x0i, y0i + 1, (1 - wx) * wy)
            + gather(x0i + 1, y0i + 1, wx * wy))


def _numpy_reference(features, deps_feature_stage, proj_matrices, depth_values,
                     reg_w):
    f32 = np.float32
    features = np.asarray(features, f32)
    deps_feature_stage = np.asarray(deps_feature_stage, f32)
    proj_matrices = np.asarray(proj_matrices, f32)
    depth_values = np.asarray(depth_values, f32)
    reg_w = np.asarray(reg_w, f32)
    num_views = features.shape[0]
    b, d, h, w = depth_values.shape
    ref_proj = proj_matrices[:, 0]

    def homo_warp(src_fea, src_proj):
        bb, c, Hs, Ws = src_fea.shape
        proj = src_proj @ np.linalg.inv(ref_proj.astype(np.float64)).astype(f32)
        rot = proj[:, :3, :3]
        tr = proj[:, :3, 3]
        ys, xs = np.meshgrid(np.arange(h, dtype=f32), np.arange(w, dtype=f32),
                             indexing="ij")
        xyz = np.stack([xs.reshape(-1), ys.reshape(-1),
                        np.ones(h * w, f32)], 0)
        rx = np.einsum("bij,jn->bin", rot, xyz).astype(f32)
        dvals = depth_values.reshape(bb, 1, d, h * w)
        pxyz = rx[:, :, None, :] * dvals + tr[:, :, None, None]
        z = pxyz[:, 2]
        px = pxyz[:, 0] / z
        py = pxyz[:, 1] / z
        out = np.stack([_bilinear_np(src_fea[i], px[i], py[i])
                        for i in range(bb)])
        return out.reshape(bb, c, d, h, w)

    def variance_volume(feats):
        ref = feats[0]
        ref_vol = np.broadcast_to(
            ref[:, :, None], (ref.shape[0], ref.shape[1], d) + ref.shape[2:])
        s = ref_vol.astype(f32).copy()
        sq = (ref_vol * ref_vol).astype(f32)
        for v in range(1, num_views):
            wv = homo_warp(feats[v], proj_matrices[:, v])
            s = s + wv
            sq = sq + wv * wv
        return sq / num_views - (s / num_views) ** 2

    volume_variance = variance_volume(features)
    deps_variance = variance_volume(deps_feature_stage)
    var = np.concatenate([volume_variance, deps_variance], axis=1)
    cost = np.einsum("bcdhw,oc->bodhw", var, reg_w)[:, 0]
    cmax = cost.max(axis=1, keepdims=True)
    e = np.exp(cost - cmax)
    prob_volume = e / e.sum(axis=1, keepdims=True)
    depth = np.sum(prob_volume * depth_values, axis=1)
    samp_variance = (depth_values - depth[:, None]) ** 2
    exp_variance = LAMB * np.sqrt(np.sum(samp_variance * prob_volume, axis=1))
    return (depth.astype(f32), exp_variance.astype(f32),
            prob_volume.astype(f32))


# --------------------------------------------------------------------------
# bass program
# --------------------------------------------------------------------------

_PROGRAM_CACHE = {}


def _build_program(H, W, C, D, kpad, ncp):
    """Build the SPMD bass program.  Per-core divergence comes only from the
    input tensors (kofs/ab tables, depth slice)."""
    key = (H, W, C, D, kpad, ncp, IMG_DT, WORK_DT, NQ)
    if key in _PROGRAM_CACHE:
        return _PROGRAM_CACHE[key]

    ND = D // N_CORES          # depth planes per core
    HS = H // N_CORES          # h rows per core (softmax phase)
    HW = HS * W                # 2560
    WPAD = W + 2 * kpad
    WQ = W // NQ               # output w per split
    FQ = WQ * C                # free elems per split window
    ncn = C - ncp

    nc = bacc.Bacc(None, num_devices=N_CORES)

    img_p = [nc.declare_dram_parameter(f"img{v}", [H, WPAD * C], IMG_DT, False)
             for v in range(3)]
    kofs_p = nc.declare_dram_parameter("kofs", [1, 2 * ND], mybir.dt.int32,
                                       False)
    ab_p = nc.declare_dram_parameter("ab", [H, 4 * ND], mybir.dt.float32, False)
    dvs_p = nc.declare_dram_parameter("dvs", [D, HW], mybir.dt.float32, False)
    prob_p = nc.declare_dram_parameter("prob", [D, HW], mybir.dt.float32, True)
    dep_p = nc.declare_dram_parameter("dep", [1, HW], mybir.dt.float32, True)
    ev_p = nc.declare_dram_parameter("ev", [1, HW], mybir.dt.float32, True)

    cost_send = nc.dram_tensor("cost_send", [H, ND * W], mybir.dt.float32)
    cost_recv = nc.dram_tensor("cost_recv", [H, ND * W], mybir.dt.float32)

    f32 = mybir.dt.float32
    AX = mybir.AxisListType
    AF = mybir.ActivationFunctionType

    with tile.TileContext(nc) as tc:
        with (
            tc.tile_pool(name="imgs", bufs=1) as imgs,
            tc.tile_pool(name="small", bufs=1) as small,
            tc.tile_pool(name="work", bufs=2) as work,
            tc.tile_pool(name="slab", bufs=1) as slabs,
            tc.tile_pool(name="soft", bufs=1) as soft,
            tc.tile_pool(name="psum", bufs=4, space="PSUM") as psum,
        ):
            # ---- loads ----
            img_t = []
            for v in range(3):
                t = imgs.tile([H, WPAD * C], IMG_DT, tag=f"img{v}")
                nc.sync.dma_start(t[:], img_p[v][:])
                img_t.append(t)
            kofs_t = small.tile([1, 2 * ND], mybir.dt.int32, tag="kofs")
            nc.sync.dma_start(kofs_t[:], kofs_p[:])
            ab_t = small.tile([H, 4 * ND], f32, tag="ab")
            nc.sync.dma_start(ab_t[:], ab_p[:])
            dvs_t = soft.tile([D, HW], f32, tag="dvs")
            nc.sync.dma_start(dvs_t[:], dvs_p[:])

            slabp = slabs.tile([H, ND * W], f32, tag="slabp")
            slabn = slabs.tile([H, ND * W], f32, tag="slabn")
            nc.vector.memset(slabp[:], 0.0)
            nc.gpsimd.memset(slabn[:], 0.0)

            # ---- phase B: cost volume for my ND planes ----
            WIN = (W + 1) * C
            for d in range(ND):
                # dynamic full-row window per view (register offset read
                # from the kofs table; bacc graph-colors the registers)
                stg = []
                for v in range(2):
                    idx = v * ND + d
                    eng = nc.vector
                    tmp = eng.alloc_register(f"kofs_{v}_{d}_{nc.next_id()}")
                    eng.reg_load(tmp, kofs_t[0:1, idx:idx + 1])
                    off = eng.snap(tmp, donate=True, min_val=0,
                                   max_val=(WPAD - W - 2) * C)
                    stg.append(img_t[v + 1][:, bass.ds(off, WIN)])
                for q in range(NQ):
                    q0 = q * WQ * C
                    wins = [(st[:, q0:q0 + FQ], st[:, q0 + C:q0 + C + FQ])
                            for st in stg]
                    refw = img_t[0][:, (kpad + q * WQ) * C:
                                    (kpad + q * WQ) * C + FQ]

                    # d1 = ref - a1*A1 - b1*B1 ; d2 = ref - a2*A2 - b2*B2
                    f1 = work.tile([H, FQ], WORK_DT, tag="f1")
                    nc.vector.affine_then_add(
                        f1[:], wins[0][0], refw,
                        scale=ab_t[:, 4 * d + 0:4 * d + 1], bias=0.0)
                    nc.vector.affine_then_add(
                        f1[:], wins[0][1], f1[:],
                        scale=ab_t[:, 4 * d + 1:4 * d + 2], bias=0.0)
                    f2 = work.tile([H, FQ], WORK_DT, tag="f2")
                    nc.vector.affine_then_add(
                        f2[:], wins[1][0], refw,
                        scale=ab_t[:, 4 * d + 2:4 * d + 3], bias=0.0)
                    nc.vector.affine_then_add(
                        f2[:], wins[1][1], f2[:],
                        scale=ab_t[:, 4 * d + 3:4 * d + 4], bias=0.0)
                    # u = d1 - 0.5*d2  (into f1)
                    nc.vector.affine_then_add(
                        f1[:], f2[:], f1[:], scale=-0.5, bias=0.0)
                    # t = u^2 + 0.75*d2^2 (squares in place, add into f1)
                    nc.scalar.square(f1[:], f1[:])
                    nc.scalar.activation(f2[:], f2[:], AF.Square,
                                         scale=0.8660254037844386)
                    nc.gpsimd.tensor_add(f1[:], f1[:], f2[:])
                    tv = f1[:].rearrange("p (w c) -> p w c", c=C)
                    dst = slice(d * W + q * WQ, d * W + q * WQ + WQ)
                    if ncp > 0:
                        nc.vector.reduce_sum(
                            out=slabp[:, dst], in_=tv[:, :, 0:ncp], axis=AX.X)
                    if ncn > 0:
                        nc.vector.reduce_sum(
                            out=slabn[:, dst], in_=tv[:, :, ncp:C], axis=AX.X)

            cost_sb = slabs.tile([H, ND * W], f32, tag="cost")
            nc.vector.tensor_sub(cost_sb[:], slabp[:], slabn[:])

            # ---- exchange: d-shard -> h-shard ----
            # AllToAll splits dim0 (128 h-rows) into 8 blocks of HS rows:
            # block g of my [HS, ND*W] goes to core g; I receive, at block r,
            # core r's planes for my h-rows.
            nc.sync.dma_start(cost_send[:], cost_sb[:])
            nc.gpsimd.collective_compute(
                "AllToAll", mybir.AluOpType.bypass,
                replica_groups=[list(range(N_CORES))],
                ins=[cost_send[:]], outs=[cost_recv[:]])
            ccols = soft.tile([D, HW], f32, tag="ccols")
            for r in range(N_CORES):
                src = cost_recv[r * HS:(r + 1) * HS, :].rearrange(
                    "hh (d w) -> d hh w", w=W)
                dst = ccols[r * ND:(r + 1) * ND, :].rearrange(
                    "d (hh w) -> d hh w", w=W)
                nc.sync.dma_start(dst, src)

            # ---- softmax over D + outputs ----
            # All row vectors live on partition 0 (HW requires compute APs
            # to start at partition 0/32/64/96).
            E = ccols  # exp in place
            nc.scalar.activation(E[:], ccols[:], AF.Exp)
            Edv = soft.tile([D, HW], f32, tag="Edv")
            nc.vector.tensor_mul(Edv[:], E[:], dvs_t[:])

            ones_d = small.tile([D, 1], f32, tag="ones_d")
            nc.vector.memset(ones_d[:], 1.0)
            ones_1 = small.tile([1, D], f32, tag="ones_1")
            nc.vector.memset(ones_1[:], 1.0)

            s0row = soft.tile([1, HW], f32, tag="s0row")
            s1row = soft.tile([1, HW], f32, tag="s1row")
            s2row = soft.tile([1, HW], f32, tag="s2row")
            nchunk = (HW + 511) // 512

            def dsum(dst_row, src):
                for ch in range(nchunk):
                    sl = slice(ch * 512, min(HW, (ch + 1) * 512))
                    n = sl.stop - sl.start
                    pt = psum.tile([1, 512], f32, tag="spsum")
                    nc.tensor.matmul(pt[:, 0:n], ones_d[:], src[:, sl],
                                     start=True, stop=True)
                    nc.vector.tensor_copy(dst_row[0:1, sl], pt[:, 0:n])

            dsum(s0row, E)
            dsum(s1row, Edv)
            # Edv2 reuses Edv in place (S1 already summed)
            nc.vector.tensor_mul(Edv[:], Edv[:], dvs_t[:])
            dsum(s2row, Edv)

            r0 = soft.tile([1, HW], f32, tag="r0")
            dep_row = soft.tile([1, HW], f32, tag="dep_row")
            nc.vector.reciprocal(r0[:], s0row[:])
            nc.vector.tensor_mul(dep_row[:], s1row[:], r0[:])
            m2 = s0row  # dead, reuse
            nc.vector.tensor_mul(m2[:], s2row[:], r0[:])
            dsq = s1row  # dead, reuse
            nc.vector.tensor_mul(dsq[:], dep_row[:], dep_row[:])
            ev2 = s2row  # dead, reuse
            nc.vector.tensor_sub(ev2[:], m2[:], dsq[:])
            ev_row = m2  # dead, reuse
            # 1.5*sqrt(x) = sqrt(2.25*x)
            nc.scalar.activation(ev_row[:], ev2[:], AF.Sqrt, scale=2.25)
            nc.sync.dma_start(dep_p[:], dep_row[:])
            nc.sync.dma_start(ev_p[:], ev_row[:])

            # prob = E * (1/S0) broadcast over D (reuse Edv as the buffer)
            prob_sb = Edv
            for ch in range(nchunk):
                sl = slice(ch * 512, min(HW, (ch + 1) * 512))
                n = sl.stop - sl.start
                pb = psum.tile([D, 512], f32, tag="bpsum")
                nc.tensor.matmul(pb[:, 0:n], ones_1[:], r0[:, sl],
                                 start=True, stop=True)
                nc.vector.tensor_mul(prob_sb[:, sl], E[:, sl], pb[:, 0:n])
            nc.sync.dma_start(prob_p[:], prob_sb[:])

    if not nc.is_finalized():
        nc.finalize()
    _PROGRAM_CACHE[key] = nc
    return nc


# --------------------------------------------------------------------------
# entry point
# --------------------------------------------------------------------------

def kernel(features, deps_feature_stage, proj_matrices, depth_values, reg_w,
           num_depth=None, stage_idx=None, **_unused):
    features = np.ascontiguousarray(np.asarray(features, np.float32))
    deps_feature_stage = np.ascontiguousarray(
        np.asarray(deps_feature_stage, np.float32))
    proj_matrices = np.asarray(proj_matrices, np.float32)
    depth_values = np.ascontiguousarray(np.asarray(depth_values, np.float32))
    reg_w = np.asarray(reg_w, np.float32)

    V, B, C, H, W = features.shape
    CD = deps_feature_stage.shape[2]
    D = depth_values.shape[1]
    CT = C + CD

    ok = (V == 3 and B == 1 and H % N_CORES == 0 and D % N_CORES == 0
          and W % NQ == 0 and proj_matrices.shape == (1, 3, 4, 4)
          and reg_w.shape == (1, CT))
    shifts = _analyze_shifts(proj_matrices, depth_values, H, W) if ok else None
    if shifts is None:
        return _numpy_reference(features, deps_feature_stage, proj_matrices,
                                depth_values, reg_w)

    # ---- host prep ----
    w_vec = reg_w[0].astype(np.float64)
    perm = np.argsort(w_vec < 0, kind="stable")  # positives first
    ncp = int(np.sum(w_vec >= 0))
    scale = np.sqrt(2.0 * np.abs(w_vec[perm]) / 9.0).astype(np.float32)

    kmax = max(int(np.max(np.abs(s[0]))) for s in shifts)
    kmax = min(kmax, W + 2)
    kpad = kmax + 2
    WPAD = W + 2 * kpad
    ND = D // N_CORES
    HS = H // N_CORES
    WQ = W // NQ

    # images: concat channels, permute+scale, zero-pad w, transpose to
    # [H, WPAD, CT] then flatten free dims
    if IMG_DT == mybir.dt.float32:
        np_img_dt = np.float32
    else:
        import ml_dtypes
        np_img_dt = ml_dtypes.bfloat16
    imgs = []
    for v in range(3):
        cat = np.concatenate([features[v, 0], deps_feature_stage[v, 0]],
                             axis=0)  # [CT,H,W]
        cat = cat[perm] * scale[:, None, None]
        pad = np.zeros((CT, H, WPAD), np.float32)
        pad[:, :, kpad:kpad + W] = cat
        imgT = np.ascontiguousarray(
            pad.transpose(1, 2, 0).reshape(H, WPAD * CT)).astype(np_img_dt)
        imgs.append(imgT)

    # clamp shifts so windows stay inside the padded image; a plane whose
    # window is fully outside contributes zeros either way
    def clamp_k(k):
        return int(np.clip(k, -(kpad - 2), kpad - 2))

    in_maps = []
    for core in range(N_CORES):
        dsl = slice(core * ND, (core + 1) * ND)
        kofs = np.zeros((1, 2 * ND), np.int32)
        ab = np.zeros((H, 4 * ND), np.float32)
        for v in range(2):
            kv, fv = shifts[v]
            for dl in range(ND):
                k = clamp_k(kv[dsl][dl])
                frac = float(fv[dsl][dl])
                # warp inside image: a=(1-frac), b=frac (zero-padded image
                # encodes the validity masking)
                ab[:, 4 * dl + 2 * v + 0] = -(1.0 - frac)
                ab[:, 4 * dl + 2 * v + 1] = -frac
                kofs[0, v * ND + dl] = (kpad + k) * CT
        dvs = np.ascontiguousarray(
            depth_values[0, :, core * HS:(core + 1) * HS, :].reshape(D, HS * W)
        ).astype(np.float32)
        in_maps.append({
            "img0": imgs[0], "img1": imgs[1], "img2": imgs[2],
            "kofs": kofs, "ab": ab, "dvs": dvs,
        })

    nc = _build_program(H, W, CT, D, kpad, ncp)
    global LAST_RESULT, _LAST_IN_MAPS
    _LAST_IN_MAPS = in_maps
    res = run_bass_kernel_spmd(nc, in_maps, list(range(N_CORES)))
    LAST_RESULT = res

    depth = np.zeros((1, H, W), np.float32)
    exp_var = np.zeros((1, H, W), np.float32)
    prob = np.zeros((1, D, H, W), np.float32)
    for core in range(N_CORES):
        r = res.results[core]
        hsl = slice(core * HS, (core + 1) * HS)
        depth[0, hsl] = r["dep"].reshape(HS, W)
        exp_var[0, hsl] = r["ev"].reshape(HS, W)
        prob[0, :, hsl] = r["prob"].reshape(D, HS, W)
    return depth, exp_var, prob
